# revision 5
# baseline (speedup 1.0000x reference)
"""Segment-reduce BatchNorm (scalar + vector branches) on 8 TRN2 NeuronCores.

Strategy (per sharding hint): split the 512 sorted segments into 8 blocks of
64 contiguous segments; each core gets the nodes of its 64 segments, so all
segment statistics are device-local (no collectives).

Per-core device program (SPMD, identical program, per-core input values):
  Phase 1 (stats):  stream bf16-staged [s | v | one-hot P] node tiles; PE
                    matmuls P^T@s, P^T@s^2, P^T@|v|^2 accumulate per-segment
                    sums in PSUM.
  Phase 2 (final):  tiny [64,*] ops: mean/var, Newton-refined rsqrt, scale A
                    and offset C per segment, split into bf16 hi+lo halves.
  Phase 3 (apply):  stream f32 node tiles; PE expands per-node A/C/r via
                    one-hot matmuls (hi+lo accumulated in PSUM => ~fp32
                    precision); DVE computes s*A+C and v*r; DMA out.

All host work is index/layout-only (sharding, padding, one-hot build, dtype
staging); every reduction and normalization happens on device.
"""

import numpy as np
import ml_dtypes

bf16 = ml_dtypes.bfloat16

B = 512          # total segments
NCORES = 8
BL = B // NCORES  # 64 segments per core
NT = 26624       # padded nodes per core (208 tiles of 128)
T = NT // 128    # 208 node tiles
K1 = 8           # phase-1 node tiles per DMA supertile
K3 = 4           # phase-3 node tiles per DMA supertile
SDIM = 256
VD = 192         # 64 channels x 3 components, staged component-major
EPS = 1e-6

IN1_W = SDIM + VD + BL          # 512 bf16 cols per node: s16 | v16 | p1
SV_W = SDIM + VD                # 448 f32 cols per node: s | v

_compiled = None


def _emit(ctx, tc, nc, mybir, d_in1, d_p3, d_sv, d_ci, d_wb, d_out):
    import concourse.bass as bass

    f32 = mybir.dt.float32
    b16 = mybir.dt.bfloat16
    ts = bass.ts

    const_pool = ctx.enter_context(tc.tile_pool(name="const", bufs=1))
    cnti = const_pool.tile([BL, 1], f32)
    nc.sync.dma_start(cnti[:], d_ci[:])
    wbt = const_pool.tile([BL, 2 * SDIM], f32)
    nc.sync.dma_start(wbt[:], d_wb[:])

    # ---- phase 1: per-segment sums ----
    pstat = ctx.enter_context(tc.tile_pool(name="pstat", bufs=1, space="PSUM"))
    ps_s = pstat.tile([BL, SDIM], f32, tag="ps_s")    # sum s
    ps_s2 = pstat.tile([BL, SDIM], f32, tag="ps_s2")  # sum s^2
    ps_v = pstat.tile([BL, BL], f32, tag="ps_v")      # sum |v|^2 per channel

    in1_pool = ctx.enter_context(tc.tile_pool(name="in1", bufs=3))
    sc_pool = ctx.enter_context(tc.tile_pool(name="sc", bufs=3))
    S1 = T // K1
    for si in range(S1):
        t = in1_pool.tile([128, K1 * IN1_W], b16)
        nc.sync.dma_start(t[:], d_in1[:, ts(si, K1 * IN1_W)])
        for j in range(K1):
            i = si * K1 + j
            base = j * IN1_W
            s16 = t[:, base:base + SDIM]
            v16 = t[:, base + SDIM:base + SDIM + VD]
            p1 = t[:, base + SDIM + VD:base + IN1_W]
            # scratch: 0:256 s^2 | 256:320 vsq | 320:512 v^2
            sc = sc_pool.tile([128, 512], b16)
            nc.vector.tensor_mul(sc[:, 0:SDIM], s16, s16)
            nc.vector.tensor_mul(sc[:, 320:512], v16, v16)
            nc.vector.tensor_add(sc[:, 256:320], sc[:, 320:384], sc[:, 384:448])
            nc.vector.tensor_add(sc[:, 256:320], sc[:, 256:320], sc[:, 448:512])
            st = (i == 0)
            sp = (i == T - 1)
            nc.tensor.matmul(ps_s[:], p1, s16, start=st, stop=sp)
            nc.tensor.matmul(ps_s2[:], p1, sc[:, 0:SDIM], start=st, stop=sp)
            nc.tensor.matmul(ps_v[:], p1, sc[:, 256:320], start=st, stop=sp)

    # ---- phase 2: finalize per-segment scale/offset ----
    fin = ctx.enter_context(tc.tile_pool(name="fin", bufs=1))
    ci = cnti[:, 0:1]
    smean = fin.tile([BL, SDIM], f32)
    nc.vector.tensor_scalar_mul(smean[:], ps_s[:], ci)
    ex2 = fin.tile([BL, SDIM], f32)
    nc.vector.tensor_scalar_mul(ex2[:], ps_s2[:], ci)
    sm2 = fin.tile([BL, SDIM], f32)
    nc.vector.tensor_mul(sm2[:], smean[:], smean[:])
    var0 = fin.tile([BL, SDIM], f32)
    nc.vector.tensor_sub(var0[:], ex2[:], sm2[:])
    varc = fin.tile([BL, SDIM], f32)
    nc.vector.tensor_scalar_max(varc[:], var0[:], EPS)
    rsq = fin.tile([BL, SDIM], f32)
    nc.vector.reciprocal(rsq[:], varc[:])
    y = fin.tile([BL, SDIM], f32)
    nc.scalar.sqrt(y[:], rsq[:])  # loose ACT sqrt seed for 1/sqrt(var)
    tn = fin.tile([BL, SDIM], f32)
    for _ in range(2):  # Newton: y <- y*(1.5 - 0.5*var*y^2)
        nc.vector.tensor_mul(tn[:], varc[:], y[:])
        nc.vector.tensor_mul(tn[:], tn[:], y[:])
        nc.vector.tensor_scalar(tn[:], tn[:], -0.5, 1.5, mybir.AluOpType.mult,
                                mybir.AluOpType.add)
        nc.vector.tensor_mul(y[:], y[:], tn[:])
    # AC = [A | C]: A = prec*weight, C = bias - smean*A
    AC = fin.tile([BL, 2 * SDIM], f32)
    nc.vector.tensor_mul(AC[:, 0:SDIM], y[:], wbt[:, 0:SDIM])
    mA = fin.tile([BL, SDIM], f32)
    nc.vector.tensor_mul(mA[:], smean[:], AC[:, 0:SDIM])
    nc.vector.tensor_sub(AC[:, SDIM:2 * SDIM], wbt[:, SDIM:2 * SDIM], mA[:])
    rhs_hi = fin.tile([BL, 2 * SDIM], b16)
    nc.vector.tensor_copy(rhs_hi[:], AC[:])
    hf = fin.tile([BL, 2 * SDIM], f32)
    nc.vector.tensor_copy(hf[:], rhs_hi[:])
    lof = fin.tile([BL, 2 * SDIM], f32)
    nc.vector.tensor_sub(lof[:], AC[:], hf[:])
    rhs_lo = fin.tile([BL, 2 * SDIM], b16)
    nc.vector.tensor_copy(rhs_lo[:], lof[:])
    # vector branch: r = 1/max(mean |v|^2, EPS), bf16 hi/lo
    vm = fin.tile([BL, BL], f32)
    nc.vector.tensor_scalar_mul(vm[:], ps_v[:], ci)
    vmc = fin.tile([BL, BL], f32)
    nc.vector.tensor_scalar_max(vmc[:], vm[:], EPS)
    rv = fin.tile([BL, BL], f32)
    nc.vector.reciprocal(rv[:], vmc[:])
    rhs_v = fin.tile([BL, 2 * BL], b16)
    nc.vector.tensor_copy(rhs_v[:, 0:BL], rv[:])
    rhf = fin.tile([BL, BL], f32)
    nc.vector.tensor_copy(rhf[:], rhs_v[:, 0:BL])
    rlo = fin.tile([BL, BL], f32)
    nc.vector.tensor_sub(rlo[:], rv[:], rhf[:])
    nc.vector.tensor_copy(rhs_v[:, BL:2 * BL], rlo[:])

    # ---- phase 3: expand + apply ----
    sv_pool = ctx.enter_context(tc.tile_pool(name="sv", bufs=3))
    p3_pool = ctx.enter_context(tc.tile_pool(name="p3", bufs=3))
    out_pool = ctx.enter_context(tc.tile_pool(name="outp", bufs=3))
    psac_pool = ctx.enter_context(tc.tile_pool(name="psac", bufs=2, space="PSUM"))
    psr_pool = ctx.enter_context(tc.tile_pool(name="psr", bufs=2, space="PSUM"))
    S3 = T // K3
    for si in range(S3):
        tin = sv_pool.tile([128, K3 * SV_W], f32)
        nc.sync.dma_start(tin[:], d_sv[:, ts(si, K3 * SV_W)])
        tp3 = p3_pool.tile([BL, K3 * 128], b16)
        nc.sync.dma_start(tp3[:], d_p3[:, ts(si, K3 * 128)])
        tout = out_pool.tile([128, K3 * SV_W], f32)
        for j in range(K3):
            sb = j * SV_W
            s_sl = tin[:, sb:sb + SDIM]
            p3s = tp3[:, ts(j, 128)]
            psac = psac_pool.tile([128, 2 * SDIM], f32, tag="psac")
            nc.tensor.matmul(psac[:], p3s, rhs_hi[:], start=True, stop=False)
            nc.tensor.matmul(psac[:], p3s, rhs_lo[:], start=False, stop=True)
            psr = psr_pool.tile([128, BL], f32, tag="psr")
            nc.tensor.matmul(psr[:], p3s, rhs_v[:, 0:BL], start=True, stop=False)
            nc.tensor.matmul(psr[:], p3s, rhs_v[:, BL:2 * BL], start=False,
                             stop=True)
            so = tout[:, sb:sb + SDIM]
            nc.vector.tensor_mul(so, s_sl, psac[:, 0:SDIM])
            nc.vector.tensor_add(so, so, psac[:, SDIM:2 * SDIM])
            for k in range(3):
                c0 = sb + SDIM + k * BL
                nc.vector.tensor_mul(tout[:, c0:c0 + BL],
                                     tin[:, c0:c0 + BL], psr[:])
        nc.sync.dma_start(d_out[:, ts(si, K3 * SV_W)], tout[:])


def _build():
    import concourse.bacc as bacc
    import concourse.tile as tile
    import concourse.mybir as mybir

    nc = bacc.Bacc("TRN2", target_bir_lowering=False, debug=False,
                   num_devices=NCORES)
    d_in1 = nc.dram_tensor("in1", [128, T * IN1_W], mybir.dt.bfloat16,
                           kind="ExternalInput").ap()
    d_p3 = nc.dram_tensor("p3", [BL, T * 128], mybir.dt.bfloat16,
                          kind="ExternalInput").ap()
    d_sv = nc.dram_tensor("sv", [128, T * SV_W], mybir.dt.float32,
                          kind="ExternalInput").ap()
    d_ci = nc.dram_tensor("ci", [BL, 1], mybir.dt.float32,
                          kind="ExternalInput").ap()
    d_wb = nc.dram_tensor("wb", [BL, 2 * SDIM], mybir.dt.float32,
                          kind="ExternalInput").ap()
    d_out = nc.dram_tensor("out", [128, T * SV_W], mybir.dt.float32,
                           kind="ExternalOutput").ap()
    from contextlib import ExitStack
    with tile.TileContext(nc) as tc:
        with ExitStack() as ctx:
            _emit(ctx, tc, nc, mybir, d_in1, d_p3, d_sv, d_ci, d_wb, d_out)
    nc.compile()
    return nc


def _get_compiled():
    global _compiled
    if _compiled is None:
        _compiled = _build()
    return _compiled


def _part_major(a, width):
    # [NT, width] node-major -> [128, T*width] partition-major supertile layout
    return np.ascontiguousarray(
        a.reshape(T, 128, width).transpose(1, 0, 2)).reshape(128, T * width)


LAST_RESULTS = None  # BassKernelResults of the most recent run (for profiling)


def prepare(s, v, batch, weight, bias):
    """Host-side sharding/staging. Returns (in_maps, metas)."""
    s = np.ascontiguousarray(np.asarray(s, dtype=np.float32))
    v = np.ascontiguousarray(np.asarray(v, dtype=np.float32))
    batch = np.asarray(batch).astype(np.int64)
    weight = np.asarray(weight, dtype=np.float32)
    bias = np.asarray(bias, dtype=np.float32)

    starts = np.searchsorted(batch, np.arange(0, B + 1, BL))
    cnt = np.bincount(batch, minlength=B).astype(np.float32)
    cnt_inv = (1.0 / np.maximum(cnt, 1.0)).astype(np.float32)
    wb = np.concatenate([np.tile(weight.reshape(1, SDIM), (BL, 1)),
                         np.tile(bias.reshape(1, SDIM), (BL, 1))],
                        axis=1).astype(np.float32)

    in_maps = []
    metas = []
    for c in range(NCORES):
        lo, hi = int(starts[c]), int(starts[c + 1])
        n = hi - lo
        assert n <= NT, f"core {c} shard {n} exceeds padded capacity {NT}"
        s16 = np.zeros((NT, SDIM), dtype=bf16)
        s16[:n] = s[lo:hi].astype(bf16)
        vp = np.zeros((NT, VD), dtype=np.float32)
        vp[:n] = v[lo:hi].transpose(0, 2, 1).reshape(n, VD)  # component-major
        v16 = vp.astype(bf16)
        segl = (batch[lo:hi] - c * BL).astype(np.int64)
        p1 = np.zeros((NT, BL), dtype=bf16)
        p1[np.arange(n), segl] = 1
        in1 = _part_major(np.concatenate([s16, v16, p1], axis=1), IN1_W)
        sf = np.zeros((NT, SDIM), dtype=np.float32)
        sf[:n] = s[lo:hi]
        sv = _part_major(np.concatenate([sf, vp], axis=1), SV_W)
        p3 = np.ascontiguousarray(
            p1.reshape(T, 128, BL).transpose(2, 0, 1)).reshape(BL, T * 128)
        ci = cnt_inv[c * BL:(c + 1) * BL].reshape(BL, 1)
        in_maps.append({"in1": in1, "p3": p3, "sv": sv, "ci": ci, "wb": wb})
        metas.append((lo, n))
    return in_maps, metas


def gather(outs, metas, N):
    """Reassemble full outputs from per-core 'out' arrays."""
    sout = np.empty((N, SDIM), dtype=np.float32)
    vout = np.empty((N, VD // 3, 3), dtype=np.float32)
    for c, (lo, n) in enumerate(metas):
        o = np.asarray(outs[c])
        o = o.reshape(128, T, SV_W).transpose(1, 0, 2).reshape(NT, SV_W)
        sout[lo:lo + n] = o[:n, 0:SDIM]
        vout[lo:lo + n] = o[:n, SDIM:SV_W].reshape(n, 3, VD // 3).transpose(0, 2, 1)
    return sout, vout


def kernel(s, v, batch, weight, bias):
    N = np.asarray(s).shape[0]
    in_maps, metas = prepare(s, v, batch, weight, bias)
    nc = _get_compiled()
    from concourse.bass_utils import run_bass_kernel_spmd
    res = run_bass_kernel_spmd(nc, in_maps, core_ids=list(range(NCORES)))
    global LAST_RESULTS
    LAST_RESULTS = res
    return gather([res.results[c]["out"] for c in range(NCORES)], metas, N)
